# revision 41
# baseline (speedup 1.0000x reference)
"""Trainium2 Bass kernel for sparse (causal, tanh-clamped) attention.

Problem: B=2, L=2048, D=1024, H=16 heads x 64 dim; S = QK^T/8;
S = 30*tanh(S); causal + attention_mask; softmax; out = attn @ V.

Sharding: 2 heads per core across 8 cores (tensor-parallel on heads).
Each core computes its 128 output features for the full batch.

Key design points:
 - All matmuls run in float32r (TF32-like, 1 cyc/row on PE for moving
   dim >= 256; HW rounds fp32 inputs internally).
 - Everything is computed in the transposed layout S^T[k, q] so that no
   P-matrix transpose is needed: S^T = K_aug^T @ Q_aug with the
   contraction (d) on partitions; the softmax numerator P^T feeds the
   AV matmul directly as the moving operand.
 - attention_mask is folded into the score matmul via an augmented 65th
   contraction row: K row 64 = (mask-1)*1e6, Q row 64 = 1.  tanh then
   saturates masked scores to -1 -> P = e^-60 ~ 0.
 - Bounded logits (30*tanh in [-30, 30]) mean softmax needs no running
   max: P = exp(30*tanh(s) - 30) in (0, 1]; the denominator comes for
   free as a ones-column appended to V in the AV matmul.
 - Causal masking: per k-tile the q range starts at the diagonal; only
   the 128x128 diagonal block needs a triu multiply on P.
 - ACT (tanh+exp, the bottleneck engine) runs on wide strips (up to
   1024 columns); tanh is computed in place in PSUM (cheaper ACT
   access).  Projections pack Q|K|V for a 256-token chunk into a
   single 2-bank PSUM slot so they interleave with attention instead
   of starving it; batch 0's attention overlaps batch 1's projections.
"""

import sys

if "/opt/trn_rl_repo" not in sys.path:
    sys.path.insert(0, "/opt/trn_rl_repo")

import numpy as np

B = 2
L = 2048
D = 1024
H = 16
DH = 64
N_CORES = 8
T = B * L            # 4096 tokens
E = 128              # per-core output features (2 heads)
NEG_BIG = 1.0e6      # mask additive; tanh saturates anything big
TAU = 30.0

_CACHE = {}


def _build_program():
    import concourse.bacc as bacc
    import concourse.tile as tile
    from concourse import mybir

    F32 = mybir.dt.float32
    F32R = mybir.dt.float32r
    AF = mybir.ActivationFunctionType

    nc = bacc.Bacc("TRN2", target_bir_lowering=False, debug=False,
                   num_devices=N_CORES)

    xT_d = nc.dram_tensor("xT", [D, T], F32R, kind="ExternalInput")
    wq_d = nc.dram_tensor("wq", [D, E], F32R, kind="ExternalInput")
    wk_d = nc.dram_tensor("wk", [D, E], F32R, kind="ExternalInput")
    wv_d = nc.dram_tensor("wv", [D, E], F32R, kind="ExternalInput")
    kaug_d = nc.dram_tensor("kaug", [1, T], F32R, kind="ExternalInput")
    ones_d = nc.dram_tensor("onesrow", [1, T], F32R, kind="ExternalInput")
    onescol_d = nc.dram_tensor("onescol", [128, 1], F32R, kind="ExternalInput")
    tril_d = nc.dram_tensor("tril", [128, 128], F32, kind="ExternalInput")
    ident_d = nc.dram_tensor("ident", [128, 128], F32R, kind="ExternalInput")
    out_d = nc.dram_tensor("out", [B, L, E], F32, kind="ExternalOutput")

    ND = D // 128        # 8 contraction chunks for projections
    NTB = L // 512       # 4 512-token groups per batch
    NK = L // 128        # 16 k tiles per sequence
    QH = 1024            # attention q-half width

    with tile.TileContext(nc) as tc:
        with (
            tc.tile_pool(name="const", bufs=1) as constp,
            tc.tile_pool(name="weights", bufs=1) as wp,
            tc.tile_pool(name="qkv", bufs=1) as qkvp,
            tc.tile_pool(name="xin", bufs=12) as xp,
            tc.tile_pool(name="work", bufs=3) as workp,
            tc.tile_pool(name="vaug", bufs=36) as vaugp,
            tc.tile_pool(name="epi", bufs=3) as epip,
            tc.tile_pool(name="ostage", bufs=32) as ostagep,
            tc.tile_pool(name="strip", bufs=3, space="PSUM") as stripp,
            tc.tile_pool(name="psO", bufs=1, space="PSUM") as psOp,
        ):
            tril_t = constp.tile([128, 128], F32, tag="tril")
            ident_t = constp.tile([128, 128], F32R, tag="ident")
            onescol_t = constp.tile([128, 1], F32R, tag="onescol")
            n30_t = constp.tile([128, 1], F32, tag="n30")
            nc.gpsimd.memset(n30_t[:], -TAU)
            identf_t = constp.tile([128, 128], F32, tag="identf")

            # weight tiles: w[:, d*128:(d+1)*128] = W.T chunk d ([128, 128])
            w_tiles = []
            for name, d_in in (("wq", wq_d), ("wk", wk_d), ("wv", wv_d)):
                wt = wp.tile([128, ND * E], F32R, tag=name, name=name)
                nc.sync.dma_start(
                    wt[:].rearrange("p (d e) -> p d e", d=ND),
                    d_in.ap().rearrange("(d p) e -> p d e", p=128),
                )
                w_tiles.append(wt)
            nc.sync.dma_start(ident_t[:], ident_d.ap()[:])
            nc.sync.dma_start(tril_t[:], tril_d.ap()[:])

            # Per (head, batch) QKV storage; row 64 = augmentation row.
            QT = [[qkvp.tile([65, L], F32R, tag=f"qt{h}{b}", name=f"qt{h}{b}")
                   for b in range(B)] for h in range(2)]
            KT = [[qkvp.tile([65, L], F32R, tag=f"kt{h}{b}", name=f"kt{h}{b}")
                   for b in range(B)] for h in range(2)]
            VT = [[qkvp.tile([64, L], F32R, tag=f"vt{h}{b}", name=f"vt{h}{b}")
                   for b in range(B)] for h in range(2)]
            def load_aug_rows(h, b):
                sl = slice(b * L, (b + 1) * L)
                nc.sync.dma_start(QT[h][b][64:65, :], ones_d.ap()[0:1, sl])
                nc.sync.dma_start(KT[h][b][64:65, :], kaug_d.ap()[0:1, sl])

            def project_group_loads(b, tp):
                g0 = b * L + tp * 512
                xts = []
                for d in range(ND):
                    xt = xp.tile([128, 512], F32R, tag="xt", name="xt")
                    nc.sync.dma_start(
                        xt[:], xT_d.ap()[d * 128:(d + 1) * 128,
                                         g0:g0 + 512])
                    xts.append(xt)
                return xts

            def project_group(b, tp, act_drains=False):
                """QKV projections for one 512-token group of batch b.

                Q|K|V for a 256-token chunk pack into ONE 2-bank strip
                slot, so a projection in flight holds a single PSUM
                slot and can interleave with attention.  Drains go to
                ACT when it is known-idle (prologue), else DVE.
                """
                xts = project_group_loads(b, tp)
                for half in (0, 1):
                    project_group_half(b, tp, xts, half, act_drains)

            def project_group_half(b, tp, xts, half, act_drains=False,
                                    ps=(0, 1, 2), pj=None):
                c0 = half * 256
                t0 = tp * 512 + c0
                if pj is None:
                    pj = stripp.tile([128, 1024], F32, tag="strip",
                                     name="pj")
                for d in range(ND):
                    for p in ps:
                        # start marks a whole 2KB PSUM zero-region as
                        # pending-zero; Q (p=0) and K (p=1) share bank
                        # 0, so only Q sets start or K's start would
                        # wipe Q's partials.  K's first-touch bytes are
                        # pending-zero from Q's mark and zero-fill.
                        nc.tensor.matmul(
                            pj[:, p * 256:p * 256 + 256],
                            w_tiles[p][:, d * E:(d + 1) * E],
                            xts[d][:, c0:c0 + 256],
                            start=(d == 0 and p != 1),
                            stop=(d == ND - 1),
                        )
                dsts = (QT, KT, VT)
                for h in range(2):
                    sl = slice(h * 64, h * 64 + 64)
                    ts_ = slice(t0, t0 + 256)
                    for p in ps:
                        dst = dsts[p]
                        csl = slice(p * 256, p * 256 + 256)
                        if act_drains and dst is not VT:
                            nc.scalar.activation(dst[h][b][0:64, ts_],
                                                 pj[sl, csl],
                                                 AF.Identity)
                        else:
                            nc.vector.tensor_copy(dst[h][b][0:64, ts_],
                                                  pj[sl, csl])
                return pj

            def vaug_prologue(b, h, kis):
                """V^T -> V tiles for one unit, with a ones column."""
                vaug = []
                for ki in kis:
                    pvt = stripp.tile([128, 64], F32R, tag="strip",
                                      name="pvt")
                    nc.tensor.transpose(
                        pvt[:], VT[h][b][0:64, ki * 128:ki * 128 + 128],
                        ident_t[0:64, 0:64])
                    va = vaugp.tile([128, 65], F32R, tag="vaug", name="va")
                    nc.vector.tensor_copy(va[:, 0:64], pvt[:])
                    nc.vector.tensor_copy(va[:, 64:65], onescol_t[:])
                    vaug.append(va)
                return vaug

            def attention_span(b, h, qlo, qw, vaug, ostage, pump):
                """Causal attention for q in [qlo, qlo+qw) of one (b, h)
                unit (qw = 512 or 1024, 512-aligned).

                `pump()` emits one queued background work unit (a
                projection piece or V prologue for a later unit); it is
                called once per k-tile so PE/DVE fill gaps while ACT
                stays busy.
                """
                po = psOp.tile([65, qw], F32, tag="psO", name="po")
                kimax = (qlo + qw) // 128 - 1
                for ki in range(kimax + 1):
                    q0 = max(qlo, ki * 128)
                    w = qlo + qw - q0
                    pss = stripp.tile([128, QH], F32, tag="strip",
                                      name="pss")
                    for off in range(0, w, 512):
                        ln = min(512, w - off)
                        nc.tensor.matmul(
                            pss[:, off:off + ln],
                            KT[h][b][:, ki * 128:ki * 128 + 128],
                            QT[h][b][:, q0 + off:q0 + off + ln],
                            start=True, stop=True)
                    # tanh in place in PSUM, then exp -> SBUF f32r
                    nc.scalar.activation(pss[:, 0:w], pss[:, 0:w],
                                         AF.Tanh, scale=0.125)
                    pp = workp.tile([128, QH], F32R, tag="prob",
                                    name="pp", bufs=6)
                    nc.scalar.activation(pp[:, 0:w], pss[:, 0:w],
                                         AF.Exp, bias=n30_t[:],
                                         scale=TAU)
                    if ki * 128 >= qlo:
                        nc.vector.tensor_mul(pp[:, 0:128], pp[:, 0:128],
                                             tril_t[:])
                    # accumulate AV per 512-wide q chunk
                    for qc in range(qlo // 512, (qlo + qw) // 512):
                        c0 = qc * 512
                        if c0 + 512 <= q0:
                            continue
                        a0 = max(q0, c0)
                        ln = c0 + 512 - a0
                        nc.tensor.matmul(
                            po[:, a0 - qlo:a0 - qlo + ln],
                            vaug[ki][:],
                            pp[:, a0 - q0:a0 - q0 + ln],
                            start=(ki == 0),
                            stop=(ki == min(kimax, 4 * qc + 3)))
                    pump()

                def epilogue():
                    # transpose back, normalize, store
                    ot = epip.tile([65, qw], F32, tag="ot", name="ot")
                    nc.vector.tensor_copy(ot[:], po[:])
                    pump()
                    for j in range(qw // 128):
                        qt_ = (qlo + j * 128) // 128   # global q tile
                        pt = psOp.tile([128, 65], F32, tag="psO",
                                       name="pt")
                        nc.tensor.transpose(
                            pt[:], ot[:, j * 128:(j + 1) * 128],
                            identf_t[0:65, 0:65])
                        of = epip.tile([128, 65], F32, tag="of", name="of")
                        nc.vector.tensor_copy(of[:], pt[:])
                        rec = epip.tile([128, 1], F32, tag="rec",
                                        name="rec")
                        nc.vector.reciprocal(rec[:], of[:, 64:65])
                        nc.vector.tensor_scalar_mul(
                            ostage[qt_][:, h * 64:(h + 1) * 64],
                            of[:, 0:64], rec[:])
                        if h == 1:   # both heads done -> store
                            nc.gpsimd.dma_start(
                                out_d.ap()[b, qt_ * 128:(qt_ + 1) * 128, :],
                                ostage[qt_][:])
                epilogue()

            ostages = [[ostagep.tile([128, 128], F32, tag="ostage",
                                     name=f"os{b}_{j}")
                        for j in range(L // 128)] for b in range(B)]

            # Orchestration: emit the minimum prologue directly, queue the
            # rest as background units pumped from inside the attention
            # loops (one unit per two pump points to spread PE load).
            from collections import deque
            pending = deque()

            def pump():
                if pending:
                    pending.popleft()()

            def flush():
                while pending:
                    pending.popleft()()

            vaugs = {}

            def queue_vaug(b, h, kis):
                def unit():
                    vaugs.setdefault((b, h), []).extend(
                        vaug_prologue(b, h, kis))
                return unit

            def queue_proj(b, tp):
                """Two pump units per 512-group (finer PE granularity)."""
                shared = {}

                def unit0():
                    shared["x"] = project_group_loads(b, tp)
                    project_group_half(b, tp, shared["x"], 0)

                def unit1():
                    project_group_half(b, tp, shared["x"], 1)
                return [unit0, unit1]

            # batch-0 front half, drains on idle ACT
            xts00 = project_group_loads(0, 0)
            load_aug_rows(0, 0)
            nc.sync.dma_start(onescol_t[:], onescol_d.ap()[:])
            nc.sync.dma_start(identf_t[:].bitcast(F32R), ident_d.ap()[:])
            load_aug_rows(1, 0)
            for half in (0, 1):
                project_group_half(0, 0, xts00, half, act_drains=True)
            project_group(0, 1, act_drains=True)
            vaugs[(0, 0)] = vaug_prologue(0, 0, range(8))
            load_aug_rows(0, 1)
            load_aug_rows(1, 1)

            pending.extend(queue_proj(0, 2))
            pending.extend(queue_proj(0, 3))
            pending.append(queue_vaug(0, 0, range(8, 12)))
            pending.append(queue_vaug(0, 0, range(12, NK)))
            spans = [
                (0, 0, 0), (0, 0, 1), (0, 1, 0), (0, 1, 1),
                (1, 0, 0), (1, 0, 1), (1, 1, 0), (1, 1, 1),
            ]
            hooks = {
                1: [queue_vaug(0, 1, range(0, 8)),
                    queue_vaug(0, 1, range(8, NK))]
                   + [u for tp in range(NTB) for u in queue_proj(1, tp)],
                3: [queue_vaug(1, 0, range(0, 8)),
                    queue_vaug(1, 0, range(8, NK))],
                5: [queue_vaug(1, 1, range(0, 8)),
                    queue_vaug(1, 1, range(8, NK))],
            }
            flush_before = {1: True, 4: True, 6: True}
            for i, (b, h, qh) in enumerate(spans):
                if flush_before.get(i):
                    flush()
                for u in hooks.get(i, []):
                    pending.append(u)
                attention_span(b, h, qh * QH, QH, vaugs[(b, h)],
                               ostages[b], pump)
            flush()

    nc.compile()
    return nc


def _get_program():
    if "nc" not in _CACHE:
        _CACHE["nc"] = _build_program()
    return _CACHE["nc"]


def _prep_inputs(input, attention_mask, W_Q, W_K, W_V):
    x = np.asarray(input, dtype=np.float32).reshape(T, D)
    xT = np.ascontiguousarray(x.T)                          # [D, T]
    mask = np.asarray(attention_mask).astype(np.float32).reshape(1, T)
    kaug = (mask - 1.0) * NEG_BIG                           # 0 keep, -1e6 drop
    onesrow = np.ones((1, T), dtype=np.float32)
    onescol = np.ones((128, 1), dtype=np.float32)
    tril = np.triu(np.ones((128, 128), dtype=np.float32))   # keep[k, q] = q >= k
    ident = np.eye(128, dtype=np.float32)

    common = {
        "xT": xT, "kaug": kaug, "onesrow": onesrow, "onescol": onescol,
        "tril": tril, "ident": ident,
    }
    in_maps = []
    for c in range(N_CORES):
        sl = slice(c * E, (c + 1) * E)
        in_maps.append({
            **common,
            "wq": np.ascontiguousarray(np.asarray(W_Q, np.float32)[sl, :].T),
            "wk": np.ascontiguousarray(np.asarray(W_K, np.float32)[sl, :].T),
            "wv": np.ascontiguousarray(np.asarray(W_V, np.float32)[sl, :].T),
        })
    return in_maps


def kernel(input, attention_mask, W_Q, W_K, W_V):
    from concourse.bass_utils import run_bass_kernel_spmd

    nc = _get_program()
    in_maps = _prep_inputs(input, attention_mask, W_Q, W_K, W_V)
    res = run_bass_kernel_spmd(nc, in_maps, list(range(N_CORES)))
    return np.concatenate([res.results[c]["out"] for c in range(N_CORES)],
                          axis=2)


# revision 43
# speedup vs baseline: 1.0065x; 1.0065x over previous
"""Trainium2 Bass kernel for sparse (causal, tanh-clamped) attention.

Problem: B=2, L=2048, D=1024, H=16 heads x 64 dim; S = QK^T/8;
S = 30*tanh(S); causal + attention_mask; softmax; out = attn @ V.

Sharding: 2 heads per core across 8 cores (tensor-parallel on heads).
Each core computes its 128 output features for the full batch.

Key design points:
 - All matmuls run in float32r (TF32-like, 1 cyc/row on PE for moving
   dim >= 256; HW rounds fp32 inputs internally).
 - Everything is computed in the transposed layout S^T[k, q] so that no
   P-matrix transpose is needed: S^T = K_aug^T @ Q_aug with the
   contraction (d) on partitions; the softmax numerator P^T feeds the
   AV matmul directly as the moving operand.
 - attention_mask is folded into the score matmul via an augmented 65th
   contraction row: K row 64 = (mask-1)*1e6, Q row 64 = 1.  tanh then
   saturates masked scores to -1 -> P = e^-60 ~ 0.
 - Bounded logits (30*tanh in [-30, 30]) mean softmax needs no running
   max: P = exp(30*tanh(s) - 30) in (0, 1]; the denominator comes for
   free as a ones-column appended to V in the AV matmul.
 - Causal masking: per k-tile the q range starts at the diagonal; only
   the 128x128 diagonal block needs a triu multiply on P.
 - ACT (tanh+exp, the bottleneck engine) runs on wide strips (up to
   1024 columns); tanh is computed in place in PSUM (cheaper ACT
   access).  Projections pack Q|K|V for a 256-token chunk into a
   single 2-bank PSUM slot so they interleave with attention instead
   of starving it; batch 0's attention overlaps batch 1's projections.
"""

import sys

if "/opt/trn_rl_repo" not in sys.path:
    sys.path.insert(0, "/opt/trn_rl_repo")

import numpy as np

B = 2
L = 2048
D = 1024
H = 16
DH = 64
N_CORES = 8
T = B * L            # 4096 tokens
E = 128              # per-core output features (2 heads)
NEG_BIG = 1.0e6      # mask additive; tanh saturates anything big
TAU = 30.0

_CACHE = {}


def _build_program():
    import concourse.bacc as bacc
    import concourse.tile as tile
    from concourse import mybir

    F32 = mybir.dt.float32
    F32R = mybir.dt.float32r
    AF = mybir.ActivationFunctionType

    nc = bacc.Bacc("TRN2", target_bir_lowering=False, debug=False,
                   num_devices=N_CORES)

    xT_d = nc.dram_tensor("xT", [D, T], F32R, kind="ExternalInput")
    wq_d = nc.dram_tensor("wq", [D, E], F32R, kind="ExternalInput")
    wk_d = nc.dram_tensor("wk", [D, E], F32R, kind="ExternalInput")
    wv_d = nc.dram_tensor("wv", [D, E], F32R, kind="ExternalInput")
    kaug_d = nc.dram_tensor("kaug", [1, T], F32R, kind="ExternalInput")
    ones_d = nc.dram_tensor("onesrow", [1, T], F32R, kind="ExternalInput")
    onescol_d = nc.dram_tensor("onescol", [128, 1], F32R, kind="ExternalInput")
    tril_d = nc.dram_tensor("tril", [128, 128], F32, kind="ExternalInput")
    ident_d = nc.dram_tensor("ident", [128, 128], F32R, kind="ExternalInput")
    out_d = nc.dram_tensor("out", [B, L, E], F32, kind="ExternalOutput")

    ND = D // 128        # 8 contraction chunks for projections
    NTB = L // 512       # 4 512-token groups per batch
    NK = L // 128        # 16 k tiles per sequence
    QH = 1024            # attention q-half width

    with tile.TileContext(nc) as tc:
        with (
            tc.tile_pool(name="const", bufs=1) as constp,
            tc.tile_pool(name="weights", bufs=1) as wp,
            tc.tile_pool(name="qkv", bufs=1) as qkvp,
            tc.tile_pool(name="xin", bufs=12) as xp,
            tc.tile_pool(name="work", bufs=3) as workp,
            tc.tile_pool(name="vaug", bufs=36) as vaugp,
            tc.tile_pool(name="epi", bufs=3) as epip,
            tc.tile_pool(name="ostage", bufs=32) as ostagep,
            tc.tile_pool(name="strip", bufs=3, space="PSUM") as stripp,
            tc.tile_pool(name="psO", bufs=1, space="PSUM") as psOp,
        ):
            tril_t = constp.tile([128, 128], F32, tag="tril")
            ident_t = constp.tile([128, 128], F32R, tag="ident")
            onescol_t = constp.tile([128, 1], F32R, tag="onescol")
            n30_t = constp.tile([128, 1], F32, tag="n30")
            nc.gpsimd.memset(n30_t[:], -TAU)
            identf_t = constp.tile([128, 128], F32, tag="identf")

            # weight tiles: w[:, d*128:(d+1)*128] = W.T chunk d ([128, 128])
            w_tiles = []
            for name, d_in in (("wq", wq_d), ("wk", wk_d), ("wv", wv_d)):
                wt = wp.tile([128, ND * E], F32R, tag=name, name=name)
                nc.sync.dma_start(
                    wt[:].rearrange("p (d e) -> p d e", d=ND),
                    d_in.ap().rearrange("(d p) e -> p d e", p=128),
                )
                w_tiles.append(wt)
            nc.sync.dma_start(ident_t[:], ident_d.ap()[:])
            nc.sync.dma_start(tril_t[:], tril_d.ap()[:])

            # Per (head, batch) QKV storage; row 64 = augmentation row.
            QT = [[qkvp.tile([65, L], F32R, tag=f"qt{h}{b}", name=f"qt{h}{b}")
                   for b in range(B)] for h in range(2)]
            KT = [[qkvp.tile([65, L], F32R, tag=f"kt{h}{b}", name=f"kt{h}{b}")
                   for b in range(B)] for h in range(2)]
            VT = [[qkvp.tile([64, L], F32R, tag=f"vt{h}{b}", name=f"vt{h}{b}")
                   for b in range(B)] for h in range(2)]
            def load_aug_rows(h, b):
                sl = slice(b * L, (b + 1) * L)
                nc.sync.dma_start(QT[h][b][64:65, :], ones_d.ap()[0:1, sl])
                nc.sync.dma_start(KT[h][b][64:65, :], kaug_d.ap()[0:1, sl])

            def project_group_loads(b, tp):
                g0 = b * L + tp * 512
                xts = []
                for d in range(ND):
                    xt = xp.tile([128, 512], F32R, tag="xt", name="xt")
                    nc.sync.dma_start(
                        xt[:], xT_d.ap()[d * 128:(d + 1) * 128,
                                         g0:g0 + 512])
                    xts.append(xt)
                return xts

            def project_group(b, tp, act_drains=False):
                """QKV projections for one 512-token group of batch b.

                Q|K|V for a 256-token chunk pack into ONE 2-bank strip
                slot, so a projection in flight holds a single PSUM
                slot and can interleave with attention.  Drains go to
                ACT when it is known-idle (prologue), else DVE.
                """
                xts = project_group_loads(b, tp)
                for half in (0, 1):
                    project_group_half(b, tp, xts, half, act_drains)

            def project_group_half(b, tp, xts, half, act_drains=False,
                                    ps=(0, 1, 2), pj=None):
                c0 = half * 256
                t0 = tp * 512 + c0
                if pj is None:
                    pj = stripp.tile([128, 1024], F32, tag="strip",
                                     name="pj")
                for d in range(ND):
                    for p in ps:
                        # start marks a whole 2KB PSUM zero-region as
                        # pending-zero; Q (p=0) and K (p=1) share bank
                        # 0, so only Q sets start or K's start would
                        # wipe Q's partials.  K's first-touch bytes are
                        # pending-zero from Q's mark and zero-fill.
                        nc.tensor.matmul(
                            pj[:, p * 256:p * 256 + 256],
                            w_tiles[p][:, d * E:(d + 1) * E],
                            xts[d][:, c0:c0 + 256],
                            start=(d == 0 and p != 1),
                            stop=(d == ND - 1),
                        )
                dsts = (QT, KT, VT)
                for h in range(2):
                    sl = slice(h * 64, h * 64 + 64)
                    ts_ = slice(t0, t0 + 256)
                    for p in ps:
                        dst = dsts[p]
                        csl = slice(p * 256, p * 256 + 256)
                        if act_drains and dst is not VT:
                            nc.scalar.activation(dst[h][b][0:64, ts_],
                                                 pj[sl, csl],
                                                 AF.Identity)
                        else:
                            nc.vector.tensor_copy(dst[h][b][0:64, ts_],
                                                  pj[sl, csl])
                return pj

            def vaug_prologue(b, h, kis):
                """V^T -> V tiles for one unit, with a ones column."""
                vaug = []
                for ki in kis:
                    pvt = stripp.tile([128, 64], F32R, tag="strip",
                                      name="pvt")
                    nc.tensor.transpose(
                        pvt[:], VT[h][b][0:64, ki * 128:ki * 128 + 128],
                        ident_t[0:64, 0:64])
                    va = vaugp.tile([128, 65], F32R, tag="vaug", name="va")
                    nc.vector.tensor_copy(va[:, 0:64], pvt[:])
                    nc.vector.tensor_copy(va[:, 64:65], onescol_t[:])
                    vaug.append(va)
                return vaug

            def attention_span(b, h, qlo, qw, vaug, ostage, pump):
                """Causal attention for q in [qlo, qlo+qw) of one (b, h)
                unit (qw = 512 or 1024, 512-aligned).

                `pump()` emits one queued background work unit (a
                projection piece or V prologue for a later unit); it is
                called once per k-tile so PE/DVE fill gaps while ACT
                stays busy.
                """
                po = psOp.tile([65, qw], F32, tag="psO", name="po")
                kimax = (qlo + qw) // 128 - 1
                for ki in range(kimax + 1):
                    q0 = max(qlo, ki * 128)
                    w = qlo + qw - q0
                    pss = stripp.tile([128, QH], F32, tag="strip",
                                      name="pss")
                    for off in range(0, w, 512):
                        ln = min(512, w - off)
                        nc.tensor.matmul(
                            pss[:, off:off + ln],
                            KT[h][b][:, ki * 128:ki * 128 + 128],
                            QT[h][b][:, q0 + off:q0 + off + ln],
                            start=True, stop=True)
                    # tanh in place in PSUM, then exp -> SBUF f32r
                    nc.scalar.activation(pss[:, 0:w], pss[:, 0:w],
                                         AF.Tanh, scale=0.125)
                    pp = workp.tile([128, QH], F32R, tag="prob",
                                    name="pp", bufs=6)
                    nc.scalar.activation(pp[:, 0:w], pss[:, 0:w],
                                         AF.Exp, bias=n30_t[:],
                                         scale=TAU)
                    if ki * 128 >= qlo:
                        nc.vector.tensor_mul(pp[:, 0:128], pp[:, 0:128],
                                             tril_t[:])
                    # accumulate AV per 512-wide q chunk
                    for qc in range(qlo // 512, (qlo + qw) // 512):
                        c0 = qc * 512
                        if c0 + 512 <= q0:
                            continue
                        a0 = max(q0, c0)
                        ln = c0 + 512 - a0
                        nc.tensor.matmul(
                            po[:, a0 - qlo:a0 - qlo + ln],
                            vaug[ki][:],
                            pp[:, a0 - q0:a0 - q0 + ln],
                            start=(ki == 0),
                            stop=(ki == min(kimax, 4 * qc + 3)))
                    pump()

                def epilogue():
                    # transpose back, normalize, store
                    ot = epip.tile([65, qw], F32, tag="ot", name="ot")
                    nc.vector.tensor_copy(ot[:], po[:])
                    pump()
                    for j in range(qw // 128):
                        qt_ = (qlo + j * 128) // 128   # global q tile
                        pt = psOp.tile([128, 65], F32, tag="psO",
                                       name="pt")
                        nc.tensor.transpose(
                            pt[:], ot[:, j * 128:(j + 1) * 128],
                            identf_t[0:65, 0:65])
                        of = epip.tile([128, 65], F32, tag="of", name="of")
                        nc.vector.tensor_copy(of[:], pt[:])
                        rec = epip.tile([128, 1], F32, tag="rec",
                                        name="rec")
                        nc.vector.reciprocal(rec[:], of[:, 64:65])
                        nc.vector.tensor_scalar_mul(
                            ostage[qt_][:, h * 64:(h + 1) * 64],
                            of[:, 0:64], rec[:])
                        if h == 1:   # both heads done -> store
                            nc.gpsimd.dma_start(
                                out_d.ap()[b, qt_ * 128:(qt_ + 1) * 128, :],
                                ostage[qt_][:])
                epilogue()

            ostages = [[ostagep.tile([128, 128], F32, tag="ostage",
                                     name=f"os{b}_{j}")
                        for j in range(L // 128)] for b in range(B)]

            # Orchestration: emit the minimum prologue directly, queue the
            # rest as background units pumped from inside the attention
            # loops (one unit per two pump points to spread PE load).
            from collections import deque
            pending = deque()

            def pump():
                if pending:
                    pending.popleft()()

            def flush():
                while pending:
                    pending.popleft()()

            vaugs = {}

            def queue_vaug(b, h, kis):
                def unit():
                    vaugs.setdefault((b, h), []).extend(
                        vaug_prologue(b, h, kis))
                return unit

            def queue_proj(b, tp):
                """Two pump units per 512-group (finer PE granularity)."""
                shared = {}

                def unit0():
                    shared["x"] = project_group_loads(b, tp)
                    project_group_half(b, tp, shared["x"], 0)

                def unit1():
                    project_group_half(b, tp, shared["x"], 1)
                return [unit0, unit1]

            # tokens 0:512 of batch 0 project first, drains on idle
            # ACT; the first 512-wide attention span starts right after.
            xts00 = project_group_loads(0, 0)
            load_aug_rows(0, 0)
            nc.sync.dma_start(onescol_t[:], onescol_d.ap()[:])
            nc.sync.dma_start(identf_t[:].bitcast(F32R), ident_d.ap()[:])
            load_aug_rows(1, 0)
            for half in (0, 1):
                project_group_half(0, 0, xts00, half, act_drains=True)
            vaugs[(0, 0)] = vaug_prologue(0, 0, range(4))
            load_aug_rows(0, 1)
            load_aug_rows(1, 1)

            pending.extend(queue_proj(0, 1))
            pending.append(queue_vaug(0, 0, range(4, 8)))
            attention_span(0, 0, 0, 512, vaugs[(0, 0)], ostages[0], pump)
            flush()
            pending.extend(queue_proj(0, 2))
            pending.extend(queue_proj(0, 3))
            pending.append(queue_vaug(0, 0, range(8, 12)))
            pending.append(queue_vaug(0, 0, range(12, NK)))
            attention_span(0, 0, 512, 512, vaugs[(0, 0)], ostages[0], pump)
            spans = [
                (0, 0, 1), (0, 1, 0), (0, 1, 1),
                (1, 0, 0), (1, 0, 1), (1, 1, 0), (1, 1, 1),
            ]
            hooks = {
                0: [queue_vaug(0, 1, range(0, 8)),
                    queue_vaug(0, 1, range(8, NK))]
                   + [u for tp in range(NTB) for u in queue_proj(1, tp)],
                2: [queue_vaug(1, 0, range(0, 8)),
                    queue_vaug(1, 0, range(8, NK))],
                4: [queue_vaug(1, 1, range(0, 8)),
                    queue_vaug(1, 1, range(8, NK))],
            }
            flush_before = {0: True, 3: True, 5: True}
            for i, (b, h, qh) in enumerate(spans):
                if flush_before.get(i):
                    flush()
                for u in hooks.get(i, []):
                    pending.append(u)
                attention_span(b, h, qh * QH, QH, vaugs[(b, h)],
                               ostages[b], pump)
            flush()

    nc.compile()
    return nc


def _get_program():
    if "nc" not in _CACHE:
        _CACHE["nc"] = _build_program()
    return _CACHE["nc"]


def _prep_inputs(input, attention_mask, W_Q, W_K, W_V):
    x = np.asarray(input, dtype=np.float32).reshape(T, D)
    xT = np.ascontiguousarray(x.T)                          # [D, T]
    mask = np.asarray(attention_mask).astype(np.float32).reshape(1, T)
    kaug = (mask - 1.0) * NEG_BIG                           # 0 keep, -1e6 drop
    onesrow = np.ones((1, T), dtype=np.float32)
    onescol = np.ones((128, 1), dtype=np.float32)
    tril = np.triu(np.ones((128, 128), dtype=np.float32))   # keep[k, q] = q >= k
    ident = np.eye(128, dtype=np.float32)

    common = {
        "xT": xT, "kaug": kaug, "onesrow": onesrow, "onescol": onescol,
        "tril": tril, "ident": ident,
    }
    in_maps = []
    for c in range(N_CORES):
        sl = slice(c * E, (c + 1) * E)
        in_maps.append({
            **common,
            "wq": np.ascontiguousarray(np.asarray(W_Q, np.float32)[sl, :].T),
            "wk": np.ascontiguousarray(np.asarray(W_K, np.float32)[sl, :].T),
            "wv": np.ascontiguousarray(np.asarray(W_V, np.float32)[sl, :].T),
        })
    return in_maps


def kernel(input, attention_mask, W_Q, W_K, W_V):
    from concourse.bass_utils import run_bass_kernel_spmd

    nc = _get_program()
    in_maps = _prep_inputs(input, attention_mask, W_Q, W_K, W_V)
    res = run_bass_kernel_spmd(nc, in_maps, list(range(N_CORES)))
    return np.concatenate([res.results[c]["out"] for c in range(N_CORES)],
                          axis=2)


# revision 44
# speedup vs baseline: 1.0247x; 1.0181x over previous
"""Trainium2 Bass kernel for sparse (causal, tanh-clamped) attention.

Problem: B=2, L=2048, D=1024, H=16 heads x 64 dim; S = QK^T/8;
S = 30*tanh(S); causal + attention_mask; softmax; out = attn @ V.

Sharding: 2 heads per core across 8 cores (tensor-parallel on heads).
Each core computes its 128 output features for the full batch.

Key design points:
 - All matmuls run in float32r (TF32-like, 1 cyc/row on PE for moving
   dim >= 256; HW rounds fp32 inputs internally).
 - Everything is computed in the transposed layout S^T[k, q] so that no
   P-matrix transpose is needed: S^T = K_aug^T @ Q_aug with the
   contraction (d) on partitions; the softmax numerator P^T feeds the
   AV matmul directly as the moving operand.
 - attention_mask is folded into the score matmul via an augmented 65th
   contraction row: K row 64 = (mask-1)*1e6, Q row 64 = 1.  tanh then
   saturates masked scores to -1 -> P = e^-60 ~ 0.
 - Bounded logits (30*tanh in [-30, 30]) mean softmax needs no running
   max: P = exp(30*tanh(s) - 30) in (0, 1]; the denominator comes for
   free as a ones-column appended to V in the AV matmul.
 - Causal masking: per k-tile the q range starts at the diagonal; only
   the 128x128 diagonal block needs a triu multiply on P.
 - ACT (tanh+exp, the bottleneck engine) runs on wide strips (up to
   1024 columns); tanh is computed in place in PSUM (cheaper ACT
   access).  Projections pack Q|K|V for a 256-token chunk into a
   single 2-bank PSUM slot so they interleave with attention instead
   of starving it; batch 0's attention overlaps batch 1's projections.
"""

import sys

if "/opt/trn_rl_repo" not in sys.path:
    sys.path.insert(0, "/opt/trn_rl_repo")

import numpy as np

B = 2
L = 2048
D = 1024
H = 16
DH = 64
N_CORES = 8
T = B * L            # 4096 tokens
E = 128              # per-core output features (2 heads)
NEG_BIG = 1.0e6      # mask additive; tanh saturates anything big
TAU = 30.0

_CACHE = {}


def _build_program():
    import concourse.bacc as bacc
    import concourse.tile as tile
    from concourse import mybir

    F32 = mybir.dt.float32
    F32R = mybir.dt.float32r
    AF = mybir.ActivationFunctionType

    nc = bacc.Bacc("TRN2", target_bir_lowering=False, debug=False,
                   num_devices=N_CORES)

    xT_d = nc.dram_tensor("xT", [D, T], F32R, kind="ExternalInput")
    wq_d = nc.dram_tensor("wq", [D, E], F32R, kind="ExternalInput")
    wk_d = nc.dram_tensor("wk", [D, E], F32R, kind="ExternalInput")
    wv_d = nc.dram_tensor("wv", [D, E], F32R, kind="ExternalInput")
    kaug_d = nc.dram_tensor("kaug", [1, T], F32R, kind="ExternalInput")
    ones_d = nc.dram_tensor("onesrow", [1, T], F32R, kind="ExternalInput")
    onescol_d = nc.dram_tensor("onescol", [128, 1], F32R, kind="ExternalInput")
    tril_d = nc.dram_tensor("tril", [128, 128], F32, kind="ExternalInput")
    ident_d = nc.dram_tensor("ident", [128, 128], F32R, kind="ExternalInput")
    out_d = nc.dram_tensor("out", [B, L, E], F32, kind="ExternalOutput")

    ND = D // 128        # 8 contraction chunks for projections
    NTB = L // 512       # 4 512-token groups per batch
    NK = L // 128        # 16 k tiles per sequence
    QH = 1024            # attention q-half width

    with tile.TileContext(nc) as tc:
        with (
            tc.tile_pool(name="const", bufs=1) as constp,
            tc.tile_pool(name="weights", bufs=1) as wp,
            tc.tile_pool(name="qkv", bufs=1) as qkvp,
            tc.tile_pool(name="xin", bufs=12) as xp,
            tc.tile_pool(name="work", bufs=3) as workp,
            tc.tile_pool(name="vaug", bufs=36) as vaugp,
            tc.tile_pool(name="epi", bufs=3) as epip,
            tc.tile_pool(name="ostage", bufs=32) as ostagep,
            tc.tile_pool(name="strip", bufs=3, space="PSUM") as stripp,
            tc.tile_pool(name="psO", bufs=1, space="PSUM") as psOp,
        ):
            tril_t = constp.tile([128, 128], F32, tag="tril")
            ident_t = constp.tile([128, 128], F32R, tag="ident")
            onescol_t = constp.tile([128, 1], F32R, tag="onescol")
            n30_t = constp.tile([128, 1], F32, tag="n30")
            nc.gpsimd.memset(n30_t[:], -TAU)
            identf_t = constp.tile([128, 128], F32, tag="identf")

            # weight tiles: w[:, d*128:(d+1)*128] = W.T chunk d ([128, 128])
            w_tiles = []
            for name, d_in in (("wq", wq_d), ("wk", wk_d), ("wv", wv_d)):
                wt = wp.tile([128, ND * E], F32R, tag=name, name=name)
                nc.sync.dma_start(
                    wt[:].rearrange("p (d e) -> p d e", d=ND),
                    d_in.ap().rearrange("(d p) e -> p d e", p=128),
                )
                w_tiles.append(wt)
            nc.sync.dma_start(ident_t[:], ident_d.ap()[:])
            nc.sync.dma_start(tril_t[:], tril_d.ap()[:])

            # Per (head, batch) QKV storage; row 64 = augmentation row.
            QT = [[qkvp.tile([65, L], F32R, tag=f"qt{h}{b}", name=f"qt{h}{b}")
                   for b in range(B)] for h in range(2)]
            KT = [[qkvp.tile([65, L], F32R, tag=f"kt{h}{b}", name=f"kt{h}{b}")
                   for b in range(B)] for h in range(2)]
            VT = [[qkvp.tile([64, L], F32R, tag=f"vt{h}{b}", name=f"vt{h}{b}")
                   for b in range(B)] for h in range(2)]
            def load_aug_rows(h, b):
                sl = slice(b * L, (b + 1) * L)
                nc.sync.dma_start(QT[h][b][64:65, :], ones_d.ap()[0:1, sl])
                nc.sync.dma_start(KT[h][b][64:65, :], kaug_d.ap()[0:1, sl])

            def project_group_loads(b, tp):
                g0 = b * L + tp * 512
                xts = []
                for d in range(ND):
                    xt = xp.tile([128, 512], F32R, tag="xt", name="xt")
                    nc.sync.dma_start(
                        xt[:], xT_d.ap()[d * 128:(d + 1) * 128,
                                         g0:g0 + 512])
                    xts.append(xt)
                return xts

            def project_group(b, tp, act_drains=False):
                """QKV projections for one 512-token group of batch b.

                Q|K|V for a 256-token chunk pack into ONE 2-bank strip
                slot, so a projection in flight holds a single PSUM
                slot and can interleave with attention.  Drains go to
                ACT when it is known-idle (prologue), else DVE.
                """
                xts = project_group_loads(b, tp)
                for half in (0, 1):
                    project_group_half(b, tp, xts, half, act_drains)

            def project_group_half(b, tp, xts, half, act_drains=False,
                                    ps=(0, 1, 2), pj=None):
                c0 = half * 256
                t0 = tp * 512 + c0
                if pj is None:
                    pj = stripp.tile([128, 1024], F32, tag="strip",
                                     name="pj")
                for d in range(ND):
                    for p in ps:
                        # start marks a whole 2KB PSUM zero-region as
                        # pending-zero; Q (p=0) and K (p=1) share bank
                        # 0, so only Q sets start or K's start would
                        # wipe Q's partials.  K's first-touch bytes are
                        # pending-zero from Q's mark and zero-fill.
                        nc.tensor.matmul(
                            pj[:, p * 256:p * 256 + 256],
                            w_tiles[p][:, d * E:(d + 1) * E],
                            xts[d][:, c0:c0 + 256],
                            start=(d == 0 and p != 1),
                            stop=(d == ND - 1),
                        )
                dsts = (QT, KT, VT)
                for h in range(2):
                    sl = slice(h * 64, h * 64 + 64)
                    ts_ = slice(t0, t0 + 256)
                    for p in ps:
                        dst = dsts[p]
                        csl = slice(p * 256, p * 256 + 256)
                        if act_drains and dst is not VT:
                            nc.scalar.activation(dst[h][b][0:64, ts_],
                                                 pj[sl, csl],
                                                 AF.Identity)
                        else:
                            nc.vector.tensor_copy(dst[h][b][0:64, ts_],
                                                  pj[sl, csl])
                return pj

            def vaug_prologue(b, h, kis):
                """V^T -> V tiles for one unit, with a ones column."""
                vaug = []
                for ki in kis:
                    pvt = stripp.tile([128, 64], F32R, tag="strip",
                                      name="pvt")
                    nc.tensor.transpose(
                        pvt[:], VT[h][b][0:64, ki * 128:ki * 128 + 128],
                        ident_t[0:64, 0:64])
                    va = vaugp.tile([128, 65], F32R, tag="vaug", name="va")
                    nc.vector.tensor_copy(va[:, 0:64], pvt[:])
                    nc.vector.tensor_copy(va[:, 64:65], onescol_t[:])
                    vaug.append(va)
                return vaug

            def attention_span(b, h, qlo, qw, vaug, ostage, pump):
                """Causal attention for q in [qlo, qlo+qw) of one (b, h)
                unit (qw = 512 or 1024, 512-aligned).

                `pump()` emits one queued background work unit (a
                projection piece or V prologue for a later unit); it is
                called once per k-tile so PE/DVE fill gaps while ACT
                stays busy.
                """
                po = psOp.tile([65, qw], F32, tag="psO", name="po")
                epilogue_half = make_epilogue(b, h, qlo, po, ostage)
                kimax = (qlo + qw) // 128 - 1
                for ki in range(kimax + 1):
                    q0 = max(qlo, ki * 128)
                    w = qlo + qw - q0
                    pss = stripp.tile([128, QH], F32, tag="strip",
                                      name="pss")
                    for off in range(0, w, 512):
                        ln = min(512, w - off)
                        nc.tensor.matmul(
                            pss[:, off:off + ln],
                            KT[h][b][:, ki * 128:ki * 128 + 128],
                            QT[h][b][:, q0 + off:q0 + off + ln],
                            start=True, stop=True)
                    # tanh in place in PSUM, then exp -> SBUF f32r
                    nc.scalar.activation(pss[:, 0:w], pss[:, 0:w],
                                         AF.Tanh, scale=0.125)
                    pp = workp.tile([128, QH], F32R, tag="prob",
                                    name="pp", bufs=6)
                    nc.scalar.activation(pp[:, 0:w], pss[:, 0:w],
                                         AF.Exp, bias=n30_t[:],
                                         scale=TAU)
                    if ki * 128 >= qlo:
                        nc.vector.tensor_mul(pp[:, 0:128], pp[:, 0:128],
                                             tril_t[:])
                    # accumulate AV per 512-wide q chunk
                    for qc in range(qlo // 512, (qlo + qw) // 512):
                        c0 = qc * 512
                        if c0 + 512 <= q0:
                            continue
                        a0 = max(q0, c0)
                        ln = c0 + 512 - a0
                        nc.tensor.matmul(
                            po[:, a0 - qlo:a0 - qlo + ln],
                            vaug[ki][:],
                            pp[:, a0 - q0:a0 - q0 + ln],
                            start=(ki == 0),
                            stop=(ki == min(kimax, 4 * qc + 3)))
                    pump()
                    if qw == 1024 and ki == 4 * (qlo // 512) + 3:
                        # first 512 columns of po are final: drain them
                        # while the remaining k-tiles accumulate the rest
                        epilogue_half(0)
                if qw == 1024:
                    epilogue_half(1)
                else:
                    epilogue_half(0)

            def make_epilogue(b, h, qlo, po, ostage):
                def epilogue_half(half):
                    # transpose back, normalize, store (one 512 chunk)
                    e0 = half * 512
                    ot = epip.tile([65, 512], F32, tag="ot", name="ot")
                    nc.vector.tensor_copy(ot[:], po[:, e0:e0 + 512])
                    for j in range(4):
                        qt_ = (qlo + e0 + j * 128) // 128  # global q tile
                        pt = psOp.tile([128, 65], F32, tag="psO",
                                       name="pt")
                        nc.tensor.transpose(
                            pt[:], ot[:, j * 128:(j + 1) * 128],
                            identf_t[0:65, 0:65])
                        of = epip.tile([128, 65], F32, tag="of", name="of")
                        nc.vector.tensor_copy(of[:], pt[:])
                        rec = epip.tile([128, 1], F32, tag="rec",
                                        name="rec")
                        nc.vector.reciprocal(rec[:], of[:, 64:65])
                        nc.vector.tensor_scalar_mul(
                            ostage[qt_][:, h * 64:(h + 1) * 64],
                            of[:, 0:64], rec[:])
                        if h == 1:   # both heads done -> store
                            nc.gpsimd.dma_start(
                                out_d.ap()[b, qt_ * 128:(qt_ + 1) * 128, :],
                                ostage[qt_][:])
                return epilogue_half

            ostages = [[ostagep.tile([128, 128], F32, tag="ostage",
                                     name=f"os{b}_{j}")
                        for j in range(L // 128)] for b in range(B)]

            # Orchestration: emit the minimum prologue directly, queue the
            # rest as background units pumped from inside the attention
            # loops (one unit per two pump points to spread PE load).
            from collections import deque
            pending = deque()

            def pump():
                if pending:
                    pending.popleft()()

            def flush():
                while pending:
                    pending.popleft()()

            vaugs = {}

            def queue_vaug(b, h, kis):
                def unit():
                    vaugs.setdefault((b, h), []).extend(
                        vaug_prologue(b, h, kis))
                return unit

            def queue_proj(b, tp):
                """Two pump units per 512-group (finer PE granularity)."""
                shared = {}

                def unit0():
                    shared["x"] = project_group_loads(b, tp)
                    project_group_half(b, tp, shared["x"], 0)

                def unit1():
                    project_group_half(b, tp, shared["x"], 1)
                return [unit0, unit1]

            # tokens 0:512 of batch 0 project first, drains on idle
            # ACT; the first 512-wide attention span starts right after.
            xts00 = project_group_loads(0, 0)
            load_aug_rows(0, 0)
            nc.sync.dma_start(onescol_t[:], onescol_d.ap()[:])
            nc.sync.dma_start(identf_t[:].bitcast(F32R), ident_d.ap()[:])
            load_aug_rows(1, 0)
            for half in (0, 1):
                project_group_half(0, 0, xts00, half, act_drains=True)
            vaugs[(0, 0)] = vaug_prologue(0, 0, range(4))
            load_aug_rows(0, 1)
            load_aug_rows(1, 1)

            pending.extend(queue_proj(0, 1))
            pending.append(queue_vaug(0, 0, range(4, 8)))
            attention_span(0, 0, 0, 512, vaugs[(0, 0)], ostages[0], pump)
            flush()
            pending.extend(queue_proj(0, 2))
            pending.extend(queue_proj(0, 3))
            pending.append(queue_vaug(0, 0, range(8, 12)))
            pending.append(queue_vaug(0, 0, range(12, NK)))
            attention_span(0, 0, 512, 512, vaugs[(0, 0)], ostages[0], pump)
            spans = [
                (0, 0, 1), (0, 1, 0), (0, 1, 1),
                (1, 0, 0), (1, 0, 1), (1, 1, 0), (1, 1, 1),
            ]
            hooks = {
                0: [queue_vaug(0, 1, range(0, 8)),
                    queue_vaug(0, 1, range(8, NK))]
                   + [u for tp in range(NTB) for u in queue_proj(1, tp)],
                2: [queue_vaug(1, 0, range(0, 8)),
                    queue_vaug(1, 0, range(8, NK))],
                4: [queue_vaug(1, 1, range(0, 8)),
                    queue_vaug(1, 1, range(8, NK))],
            }
            flush_before = {0: True, 3: True, 5: True}
            for i, (b, h, qh) in enumerate(spans):
                if flush_before.get(i):
                    flush()
                for u in hooks.get(i, []):
                    pending.append(u)
                attention_span(b, h, qh * QH, QH, vaugs[(b, h)],
                               ostages[b], pump)
            flush()

    nc.compile()
    return nc


def _get_program():
    if "nc" not in _CACHE:
        _CACHE["nc"] = _build_program()
    return _CACHE["nc"]


def _prep_inputs(input, attention_mask, W_Q, W_K, W_V):
    x = np.asarray(input, dtype=np.float32).reshape(T, D)
    xT = np.ascontiguousarray(x.T)                          # [D, T]
    mask = np.asarray(attention_mask).astype(np.float32).reshape(1, T)
    kaug = (mask - 1.0) * NEG_BIG                           # 0 keep, -1e6 drop
    onesrow = np.ones((1, T), dtype=np.float32)
    onescol = np.ones((128, 1), dtype=np.float32)
    tril = np.triu(np.ones((128, 128), dtype=np.float32))   # keep[k, q] = q >= k
    ident = np.eye(128, dtype=np.float32)

    common = {
        "xT": xT, "kaug": kaug, "onesrow": onesrow, "onescol": onescol,
        "tril": tril, "ident": ident,
    }
    in_maps = []
    for c in range(N_CORES):
        sl = slice(c * E, (c + 1) * E)
        in_maps.append({
            **common,
            "wq": np.ascontiguousarray(np.asarray(W_Q, np.float32)[sl, :].T),
            "wk": np.ascontiguousarray(np.asarray(W_K, np.float32)[sl, :].T),
            "wv": np.ascontiguousarray(np.asarray(W_V, np.float32)[sl, :].T),
        })
    return in_maps


def kernel(input, attention_mask, W_Q, W_K, W_V):
    from concourse.bass_utils import run_bass_kernel_spmd

    nc = _get_program()
    in_maps = _prep_inputs(input, attention_mask, W_Q, W_K, W_V)
    res = run_bass_kernel_spmd(nc, in_maps, list(range(N_CORES)))
    return np.concatenate([res.results[c]["out"] for c in range(N_CORES)],
                          axis=2)
